# revision 4
# baseline (speedup 1.0000x reference)
"""YOLO-detect head (1x1 conv + box decode) on 8 Trainium2 NeuronCores.

Data-parallel over batch: core b processes batch element b.

Per core, per level l (C channels, HW = ny*nx positions):
  p[hw, o] = sum_c x[c, hw] * w[o, c]      (o = a*89 + ch, a anchor, ch channel)
computed on the tensor engine as out = lhsT.T @ rhs with
  lhsT = x chunk  [K=128 channels, M<=128 hw]   (stationary)
  rhs  = w.T chunk [K=128 channels, N=267]      (moving)
so the PSUM result is already [hw, 267] = the output layout (no transpose).

Decode:
  sigmoid(p) is computed as 0.5*tanh(0.5*p) + 0.5 so that the only ACT table
  set ever needed is exp_and_others (holds BOTH tanh and exp) -> exactly one
  ~2.7us ACT table load for the whole kernel instead of one per
  sigmoid<->exp alternation.
  xy   = (sigmoid(p) + grid) * stride  -> (affine result * stride) + grid*stride
  wh   = exp(p) * anchor
  rest = sigmoid(p)
grid*stride and the anchor multipliers are tiny host-precomputed constant
tensors laid out to match the on-chip access patterns.
"""

import numpy as np

import concourse.bacc as bacc
import concourse.mybir as mybir
import concourse.tile as tile
from concourse.bass_utils import run_bass_kernel_spmd

F32 = mybir.dt.float32
AF = mybir.ActivationFunctionType
ALU = mybir.AluOpType

NCORES = 8
NA = 3          # anchors per level
NO = 89         # channels per anchor (80 classes + 5 + 4)
NCOL = NA * NO  # 267
TOTAL_ROWS = 25200
GROUP = 4       # full 128-row hw tiles per PSUM group (4 banks)

LEVELS = [
    dict(C=256,  W=80, HW=6400, stride=8.0,
         anchors=((10.0, 13.0), (16.0, 30.0), (33.0, 23.0)),   base=0),
    dict(C=512,  W=40, HW=1600, stride=16.0,
         anchors=((30.0, 61.0), (62.0, 45.0), (59.0, 119.0)),  base=19200),
    dict(C=1024, W=20, HW=400,  stride=32.0,
         anchors=((116.0, 90.0), (156.0, 198.0), (373.0, 326.0)), base=24000),
]


def _ntiles(HW):
    return (HW + 127) // 128


def _groups(HW):
    """[(t0, n_full_tiles, rows_per_tile)]; trailing partial tile is its own group."""
    full, rem = divmod(HW, 128)
    out = []
    t0 = 0
    while t0 < full:
        n = min(GROUP, full - t0)
        out.append((t0, n, 128))
        t0 += n
    if rem:
        out.append((full, 1, rem))
    return out


def _build_program(use_bias: bool):
    # Bacc (not raw Bass): its compile() runs move_matmul_waits_to_ldweights +
    # generate_event_semaphores, without which walrus rejects instructions
    # that carry more than one semaphore wait.
    nc = bacc.Bacc("TRN2", target_bir_lowering=False, debug=False)

    dram = {}
    for l, L in enumerate(LEVELS):
        nt = _ntiles(L["HW"])
        dram[f"x{l}"] = nc.dram_tensor(f"x{l}", (L["C"], L["HW"]), F32,
                                       kind="ExternalInput").ap()
        dram[f"wt{l}"] = nc.dram_tensor(f"wt{l}", (L["C"], NCOL), F32,
                                        kind="ExternalInput").ap()
        dram[f"gs{l}"] = nc.dram_tensor(f"gs{l}", (128, nt * 6), F32,
                                        kind="ExternalInput").ap()
        dram[f"am{l}"] = nc.dram_tensor(f"am{l}", (128, nt * 6), F32,
                                        kind="ExternalInput").ap()
        if use_bias:
            dram[f"b{l}"] = nc.dram_tensor(f"b{l}", (1, NCOL), F32,
                                           kind="ExternalInput").ap()
    y = nc.dram_tensor("y", (TOTAL_ROWS, NO), F32, kind="ExternalOutput").ap()

    with tile.TileContext(nc) as tc:
        with tc.tile_pool(name="consts", bufs=1) as cpool, \
             tc.tile_pool(name="xbuf", bufs=1) as xpool, \
             tc.tile_pool(name="obuf", bufs=4) as opool, \
             tc.tile_pool(name="ps", bufs=2, space="PSUM") as pspool:

            ones_t = None
            if use_bias:
                ones_t = cpool.tile([1, 128], F32, tag="ones", name="ones")
                nc.vector.memset(ones_t[:, :], 1.0)

            for l, L in enumerate(LEVELS):
                C, HW, stride = L["C"], L["HW"], L["stride"]
                KC = C // 128
                nt = _ntiles(HW)

                wts = []
                for kc in range(KC):
                    wt_t = cpool.tile([128, NCOL], F32, tag=f"wt{l}k{kc}",
                                      name=f"wt{l}k{kc}")
                    nc.sync.dma_start(out=wt_t[:, :],
                                      in_=dram[f"wt{l}"][kc * 128:(kc + 1) * 128, :])
                    wts.append(wt_t)

                b_t = None
                if use_bias:
                    b_t = cpool.tile([1, NCOL], F32, tag=f"b{l}", name=f"bt{l}")
                    nc.sync.dma_start(out=b_t[:, :], in_=dram[f"b{l}"][:, :])

                gs_t = cpool.tile([128, nt, NA, 2], F32, tag=f"gs{l}", name=f"gst{l}")
                nc.sync.dma_start(
                    out=gs_t[:, :, :, :],
                    in_=dram[f"gs{l}"].rearrange("p (t a c) -> p t a c", a=NA, c=2))
                am_t = cpool.tile([128, nt, NA, 2], F32, tag=f"am{l}", name=f"amt{l}")
                nc.sync.dma_start(
                    out=am_t[:, :, :, :],
                    in_=dram[f"am{l}"].rearrange("p (t a c) -> p t a c", a=NA, c=2))

                xks = []
                for kc in range(KC):
                    xk = xpool.tile([128, HW], F32, tag=f"x{l}k{kc}",
                                    name=f"xk{l}_{kc}")
                    nc.sync.dma_start(out=xk[:, :],
                                      in_=dram[f"x{l}"][kc * 128:(kc + 1) * 128, :])
                    xks.append(xk)

                for (t0, ntl, m) in _groups(HW):
                    ps = pspool.tile([128, ntl, 512], F32, tag="ps",
                                     name=f"ps{l}_{t0}")
                    psf = ps.rearrange("p g x -> p (g x)")
                    for i in range(ntl):
                        t = t0 + i
                        for kc in range(KC):
                            nc.tensor.matmul(
                                psf[0:m, i * 512:i * 512 + NCOL],
                                lhsT=xks[kc][:, t * 128:t * 128 + m],
                                rhs=wts[kc][:, :],
                                start=(kc == 0),
                                stop=(kc == KC - 1 and not use_bias),
                            )
                        if use_bias:
                            nc.tensor.matmul(
                                psf[0:m, i * 512:i * 512 + NCOL],
                                lhsT=ones_t[:, 0:m],
                                rhs=b_t[:, :],
                                start=False,
                                stop=True,
                            )

                    ot = opool.tile([128, ntl, NCOL], F32, tag="ot",
                                    name=f"ot{l}_{t0}")
                    ps_p = ps[0:m, :, 0:NCOL]
                    # t = tanh(0.5 * p); sigmoid(p) = 0.5*t + 0.5
                    nc.scalar.activation(ot[0:m], ps_p, AF.Tanh, scale=0.5)
                    otf = ot[0:m].rearrange("p g c -> p (g c)")
                    nc.vector.tensor_scalar(otf, otf, 1.0, 0.5, ALU.add, ALU.mult)
                    # wh: exp(p) (overwrites the sigmoid values on those cols)
                    ps_wh = ps_p.rearrange("p g (a c) -> p g a c", a=NA)[:, :, :, 2:4]
                    ot_wh = ot[0:m].rearrange("p g (a c) -> p g a c",
                                              a=NA)[:, :, :, 2:4]
                    nc.scalar.activation(ot_wh, ps_wh, AF.Exp)
                    nc.vector.tensor_mul(ot_wh, ot_wh, am_t[0:m, t0:t0 + ntl])
                    # xy: (sigmoid * stride) + grid*stride
                    ot_xy = ot[0:m].rearrange("p g (a c) -> p g a c",
                                              a=NA)[:, :, :, 0:2]
                    nc.vector.scalar_tensor_tensor(
                        ot_xy, ot_xy, float(stride), gs_t[0:m, t0:t0 + ntl],
                        op0=ALU.mult, op1=ALU.add)

                    for a in range(NA):
                        r0 = L["base"] + a * HW + t0 * 128
                        src = ot[0:m, :, a * NO:(a + 1) * NO]
                        dst = y[r0:r0 + ntl * m, :].rearrange("(g p) c -> p g c", p=m)
                        nc.sync.dma_start(out=dst, in_=src)
    nc.compile()
    return nc


_PROGS = {}


def _get_prog(use_bias: bool):
    if use_bias not in _PROGS:
        _PROGS[use_bias] = _build_program(use_bias)
    return _PROGS[use_bias]


def _host_consts():
    """Per-level (gs, am) host tensors, layout (128, ntiles*6)."""
    consts = []
    for L in LEVELS:
        HW, W, stride = L["HW"], L["W"], L["stride"]
        nt = _ntiles(HW)
        hw = np.arange(nt * 128)
        gx = (hw % W).astype(np.float32) * stride
        gy = (hw // W).astype(np.float32) * stride
        gx[HW:] = 0.0
        gy[HW:] = 0.0
        # gs[p, t, a, c]: c==0 -> gx, c==1 -> gy at position t*128+p
        gs = np.zeros((128, nt, NA, 2), np.float32)
        gs[:, :, :, 0] = gx.reshape(nt, 128).T[:, :, None]
        gs[:, :, :, 1] = gy.reshape(nt, 128).T[:, :, None]
        am = np.zeros((128, nt, NA, 2), np.float32)
        anc = np.asarray(L["anchors"], np.float32)  # (NA, 2)
        am[:, :, :, :] = anc[None, None, :, :]
        consts.append((np.ascontiguousarray(gs.reshape(128, nt * 6)),
                       np.ascontiguousarray(am.reshape(128, nt * 6))))
    return consts


_CONSTS = None


def _make_in_maps(xs, ws, bs, use_bias):
    global _CONSTS
    if _CONSTS is None:
        _CONSTS = _host_consts()
    wts = [np.ascontiguousarray(w.T.astype(np.float32, copy=False)) for w in ws]
    in_maps = []
    for core in range(NCORES):
        im = {}
        for l, L in enumerate(LEVELS):
            im[f"x{l}"] = np.ascontiguousarray(
                xs[l][core].reshape(L["C"], L["HW"]))
            im[f"wt{l}"] = wts[l]
            im[f"gs{l}"], im[f"am{l}"] = _CONSTS[l]
            if use_bias:
                im[f"b{l}"] = np.ascontiguousarray(
                    bs[l].reshape(1, NCOL).astype(np.float32, copy=False))
        in_maps.append(im)
    return in_maps


def _run(x0, x1, x2, w0, b0, w1, b1, w2, b2, **spmd_kwargs):
    xs = [np.asarray(x, dtype=np.float32) for x in (x0, x1, x2)]
    ws = [np.asarray(w, dtype=np.float32) for w in (w0, w1, w2)]
    bs = [np.asarray(b, dtype=np.float32) for b in (b0, b1, b2)]
    use_bias = any(np.any(b != 0) for b in bs)
    in_maps = _make_in_maps(xs, ws, bs, use_bias)
    res = run_bass_kernel_spmd(_get_prog(use_bias), in_maps,
                               core_ids=list(range(NCORES)), **spmd_kwargs)
    out = np.stack([res.results[c]["y"] for c in range(NCORES)], axis=0)
    return out.astype(np.float32, copy=False), res


def kernel(x0, x1, x2, w0, b0, w1, b1, w2, b2):
    out, _ = _run(x0, x1, x2, w0, b0, w1, b1, w2, b2)
    return out


def kernel_traced(x0, x1, x2, w0, b0, w1, b1, w2, b2):
    """Like kernel() but with NTFF tracing; returns (out, BassKernelResults)."""
    return _run(x0, x1, x2, w0, b0, w1, b1, w2, b2, trace=True)


# revision 5
# speedup vs baseline: 1.6215x; 1.6215x over previous
"""YOLO-detect head (1x1 conv + box decode) on 8 Trainium2 NeuronCores.

Data-parallel over batch: core b processes batch element b.

Per core, per level l (C channels, HW = ny*nx positions):
  p[hw, o] = sum_c x[c, hw] * w[o, c]      (o = a*89 + ch, a anchor, ch channel)
computed on the tensor engine as out = lhsT.T @ rhs with
  lhsT = x chunk  [K=128 channels, M<=128 hw]   (stationary, bf16)
  rhs  = w.T chunk [K=128 channels, N=267]      (moving, bf16)
so the PSUM result is already [hw, 267] — no on-chip transpose.

Decode:
  sigmoid(p) is computed as 0.5*tanh(0.5*p) + 0.5 so that the only ACT table
  set ever needed is exp_and_others (holds BOTH tanh and exp) -> exactly one
  ~2.7us ACT table load for the whole kernel instead of one per
  sigmoid<->exp alternation.
  xy   = sigmoid(p)*stride + grid*stride   (grid*stride precomputed on host)
  wh   = exp(p) * anchor
  rest = sigmoid(p)

Output layout: writing the natural (25200, 89) tensor costs one 356-byte DMA
packet per row (25200 tiny packets dominated the first profile at ~75% of all
DMA time). Instead each level is stored as (128, NA, R, 89) where partition p
holds rows {t*128+p : t in 0..R} contiguously — per-partition runs of R*356
bytes — and the host reassembles with a cheap numpy transpose.

Inputs x/w are cast to bf16 on host (halves HBM load traffic; the fp32
accumulate keeps the error ~4e-4 relative to output scale).
"""

import numpy as np
import ml_dtypes

import concourse.bacc as bacc
import concourse.mybir as mybir
import concourse.tile as tile
from concourse.bass_utils import run_bass_kernel_spmd

F32 = mybir.dt.float32
BF16 = mybir.dt.bfloat16
AF = mybir.ActivationFunctionType
ALU = mybir.AluOpType

NCORES = 8
NA = 3          # anchors per level
NO = 89         # channels per anchor (80 classes + 5 + 4)
NCOL = NA * NO  # 267
GROUP = 4       # full 128-row hw tiles per PSUM group (4 banks)

LEVELS = [
    dict(C=256,  W=80, HW=6400, stride=8.0,
         anchors=((10.0, 13.0), (16.0, 30.0), (33.0, 23.0))),
    dict(C=512,  W=40, HW=1600, stride=16.0,
         anchors=((30.0, 61.0), (62.0, 45.0), (59.0, 119.0))),
    dict(C=1024, W=20, HW=400,  stride=32.0,
         anchors=((116.0, 90.0), (156.0, 198.0), (373.0, 326.0))),
]


def _ntiles(HW):
    return (HW + 127) // 128


def _groups(HW):
    """[(t0, n_full_tiles, rows_per_tile)]; trailing partial tile is its own group."""
    full, rem = divmod(HW, 128)
    out = []
    t0 = 0
    while t0 < full:
        n = min(GROUP, full - t0)
        out.append((t0, n, 128))
        t0 += n
    if rem:
        out.append((full, 1, rem))
    return out


def _store_chunks(nt):
    """Split nt tiles into ~2 store chunks per anchor for DMA pipelining."""
    if nt <= 16:
        return [(0, nt)]
    h = (nt + 1) // 2
    return [(0, h), (h, nt - h)]


def _build_program(use_bias: bool):
    # Bacc (not raw Bass): its compile() runs move_matmul_waits_to_ldweights +
    # generate_event_semaphores, without which walrus rejects instructions
    # that carry more than one semaphore wait.
    nc = bacc.Bacc("TRN2", target_bir_lowering=False, debug=False)

    dram = {}
    for l, L in enumerate(LEVELS):
        nt = _ntiles(L["HW"])
        dram[f"x{l}"] = nc.dram_tensor(f"x{l}", (L["C"], L["HW"]), BF16,
                                       kind="ExternalInput").ap()
        dram[f"wt{l}"] = nc.dram_tensor(f"wt{l}", (L["C"], NCOL), BF16,
                                        kind="ExternalInput").ap()
        dram[f"gs{l}"] = nc.dram_tensor(f"gs{l}", (128, nt * 6), F32,
                                        kind="ExternalInput").ap()
        dram[f"am{l}"] = nc.dram_tensor(f"am{l}", (128, nt * 6), F32,
                                        kind="ExternalInput").ap()
        dram[f"y{l}"] = nc.dram_tensor(f"y{l}", (128, NA, nt, NO), F32,
                                       kind="ExternalOutput").ap()
        if use_bias:
            dram[f"b{l}"] = nc.dram_tensor(f"b{l}", (1, NCOL), F32,
                                           kind="ExternalInput").ap()

    with tile.TileContext(nc) as tc:
        with tc.tile_pool(name="consts", bufs=1) as cpool, \
             tc.tile_pool(name="xbuf", bufs=1) as xpool, \
             tc.tile_pool(name="obuf", bufs=1) as opool, \
             tc.tile_pool(name="ps", bufs=2, space="PSUM") as pspool:

            ones_t = None
            if use_bias:
                ones_t = cpool.tile([1, 128], BF16, tag="ones", name="ones")
                nc.vector.memset(ones_t[:, :], 1.0)

            for l, L in enumerate(LEVELS):
                C, HW, stride = L["C"], L["HW"], L["stride"]
                KC = C // 128
                nt = _ntiles(HW)

                wts = []
                for kc in range(KC):
                    wt_t = cpool.tile([128, NCOL], BF16, tag=f"wt{l}k{kc}",
                                      name=f"wt{l}k{kc}")
                    nc.sync.dma_start(out=wt_t[:, :],
                                      in_=dram[f"wt{l}"][kc * 128:(kc + 1) * 128, :])
                    wts.append(wt_t)

                b_t = None
                if use_bias:
                    b_t = cpool.tile([1, NCOL], BF16, tag=f"b{l}", name=f"bt{l}")
                    nc.gpsimd.dma_start(out=b_t[:, :], in_=dram[f"b{l}"][:, :])

                gs_t = cpool.tile([128, nt, NA, 2], F32, tag=f"gs{l}", name=f"gst{l}")
                nc.sync.dma_start(
                    out=gs_t[:, :, :, :],
                    in_=dram[f"gs{l}"].rearrange("p (t a c) -> p t a c", a=NA, c=2))
                am_t = cpool.tile([128, nt, NA, 2], F32, tag=f"am{l}", name=f"amt{l}")
                nc.sync.dma_start(
                    out=am_t[:, :, :, :],
                    in_=dram[f"am{l}"].rearrange("p (t a c) -> p t a c", a=NA, c=2))

                xks = []
                for kc in range(KC):
                    xk = xpool.tile([128, HW], BF16, tag=f"x{l}k{kc}",
                                    name=f"xk{l}_{kc}")
                    nc.sync.dma_start(out=xk[:, :],
                                      in_=dram[f"x{l}"][kc * 128:(kc + 1) * 128, :])
                    xks.append(xk)

                # whole level's decoded output stays resident; partition p row
                # (t, :) is output row hw = t*128+p
                ot = opool.tile([128, nt, NCOL], F32, tag=f"ot{l}", name=f"ot{l}")

                for (t0, ntl, m) in _groups(HW):
                    ps = pspool.tile([128, GROUP, 512], F32, tag="ps",
                                     name=f"ps{l}_{t0}")
                    psf = ps.rearrange("p g x -> p (g x)")
                    for i in range(ntl):
                        t = t0 + i
                        for kc in range(KC):
                            nc.tensor.matmul(
                                psf[0:m, i * 512:i * 512 + NCOL],
                                lhsT=xks[kc][:, t * 128:t * 128 + m],
                                rhs=wts[kc][:, :],
                                start=(kc == 0),
                                stop=(kc == KC - 1 and not use_bias),
                            )
                        if use_bias:
                            nc.tensor.matmul(
                                psf[0:m, i * 512:i * 512 + NCOL],
                                lhsT=ones_t[:, 0:m],
                                rhs=b_t[:, :],
                                start=False,
                                stop=True,
                            )

                    og = ot[0:m, t0:t0 + ntl]
                    ps_p = ps[0:m, 0:ntl, 0:NCOL]
                    # t = tanh(0.5 * p); sigmoid(p) = 0.5*t + 0.5
                    nc.scalar.activation(og, ps_p, AF.Tanh, scale=0.5)
                    ogf = og.rearrange("p g c -> p (g c)")
                    nc.vector.tensor_scalar(ogf, ogf, 1.0, 0.5, ALU.add, ALU.mult)
                    # wh: exp(p) (overwrites the sigmoid values on those cols)
                    ps_wh = ps_p.rearrange("p g (a c) -> p g a c", a=NA)[:, :, :, 2:4]
                    og_wh = og.rearrange("p g (a c) -> p g a c", a=NA)[:, :, :, 2:4]
                    nc.scalar.activation(og_wh, ps_wh, AF.Exp)
                    nc.vector.tensor_mul(og_wh, og_wh, am_t[0:m, t0:t0 + ntl])
                    # xy: sigmoid*stride + grid*stride
                    og_xy = og.rearrange("p g (a c) -> p g a c", a=NA)[:, :, :, 0:2]
                    nc.vector.scalar_tensor_tensor(
                        og_xy, og_xy, float(stride), gs_t[0:m, t0:t0 + ntl],
                        op0=ALU.mult, op1=ALU.add)

                for a in range(NA):
                    for (s0, snt) in _store_chunks(nt):
                        src = ot[:, s0:s0 + snt, a * NO:(a + 1) * NO]
                        dst = dram[f"y{l}"][:, a, s0:s0 + snt, :]
                        nc.sync.dma_start(out=dst, in_=src)
    nc.compile()
    return nc


_PROGS = {}


def _get_prog(use_bias: bool):
    if use_bias not in _PROGS:
        _PROGS[use_bias] = _build_program(use_bias)
    return _PROGS[use_bias]


def _host_consts():
    """Per-level (gs, am) host tensors, layout (128, ntiles*6)."""
    consts = []
    for L in LEVELS:
        HW, W, stride = L["HW"], L["W"], L["stride"]
        nt = _ntiles(HW)
        hw = np.arange(nt * 128)
        gx = (hw % W).astype(np.float32) * stride
        gy = (hw // W).astype(np.float32) * stride
        gx[HW:] = 0.0
        gy[HW:] = 0.0
        # gs[p, t, a, c]: c==0 -> gx, c==1 -> gy at position t*128+p
        gs = np.zeros((128, nt, NA, 2), np.float32)
        gs[:, :, :, 0] = gx.reshape(nt, 128).T[:, :, None]
        gs[:, :, :, 1] = gy.reshape(nt, 128).T[:, :, None]
        am = np.zeros((128, nt, NA, 2), np.float32)
        anc = np.asarray(L["anchors"], np.float32)  # (NA, 2)
        am[:, :, :, :] = anc[None, None, :, :]
        consts.append((np.ascontiguousarray(gs.reshape(128, nt * 6)),
                       np.ascontiguousarray(am.reshape(128, nt * 6))))
    return consts


_CONSTS = None


def _make_in_maps(xs, ws, bs, use_bias):
    global _CONSTS
    if _CONSTS is None:
        _CONSTS = _host_consts()
    wts = [np.ascontiguousarray(w.T.astype(ml_dtypes.bfloat16)) for w in ws]
    xbf = [np.ascontiguousarray(
        x.reshape(NCORES, L["C"], L["HW"]).astype(ml_dtypes.bfloat16))
        for x, L in zip(xs, LEVELS)]
    in_maps = []
    for core in range(NCORES):
        im = {}
        for l, L in enumerate(LEVELS):
            im[f"x{l}"] = xbf[l][core]
            im[f"wt{l}"] = wts[l]
            im[f"gs{l}"], im[f"am{l}"] = _CONSTS[l]
            if use_bias:
                im[f"b{l}"] = np.ascontiguousarray(
                    bs[l].reshape(1, NCOL).astype(ml_dtypes.bfloat16))
        in_maps.append(im)
    return in_maps


def _assemble(results):
    """results[core][f"y{l}"] (128, NA, R, 89) -> (NCORES, 25200, 89) fp32."""
    out = np.empty((NCORES, 25200, NO), np.float32)
    for core in range(NCORES):
        parts = []
        for l, L in enumerate(LEVELS):
            HW = L["HW"]
            nt = _ntiles(HW)
            y = results[core][f"y{l}"]  # (128, NA, nt, 89)
            y = y.transpose(1, 2, 0, 3).reshape(NA, nt * 128, NO)[:, :HW, :]
            parts.append(y.reshape(NA * HW, NO))
        out[core] = np.concatenate(parts, axis=0)
    return out


def _run(x0, x1, x2, w0, b0, w1, b1, w2, b2, **spmd_kwargs):
    xs = [np.asarray(x, dtype=np.float32) for x in (x0, x1, x2)]
    ws = [np.asarray(w, dtype=np.float32) for w in (w0, w1, w2)]
    bs = [np.asarray(b, dtype=np.float32) for b in (b0, b1, b2)]
    use_bias = any(np.any(b != 0) for b in bs)
    in_maps = _make_in_maps(xs, ws, bs, use_bias)
    res = run_bass_kernel_spmd(_get_prog(use_bias), in_maps,
                               core_ids=list(range(NCORES)), **spmd_kwargs)
    return _assemble(res.results), res


def kernel(x0, x1, x2, w0, b0, w1, b1, w2, b2):
    out, _ = _run(x0, x1, x2, w0, b0, w1, b1, w2, b2)
    return out


def kernel_traced(x0, x1, x2, w0, b0, w1, b1, w2, b2):
    """Like kernel() but with NTFF tracing; returns (out, BassKernelResults)."""
    return _run(x0, x1, x2, w0, b0, w1, b1, w2, b2, trace=True)


# revision 10
# speedup vs baseline: 3.0991x; 1.9112x over previous
"""YOLO-detect head (1x1 conv + box decode) on 8 Trainium2 NeuronCores.

Data-parallel over batch: core b processes batch element b.

Per core, per level l (C channels, HW = ny*nx positions):
  p[hw, o] = sum_c x[c, hw] * w[o, c]      (o = a*89 + ch, a anchor, ch channel)
computed on the tensor engine as out = lhsT.T @ rhs with
  lhsT = x chunk  [K=128 channels, M<=128 hw]   (stationary, bf16)
  rhs  = w.T chunk [K=128 channels, N=267]      (moving, bf16)
so the PSUM result is already [hw, 267] — no on-chip transpose.

Decode:
  sigmoid(p) is computed as 0.5*tanh(0.5*p) + 0.5 so that the only ACT table
  set ever needed is exp_and_others (holds BOTH tanh and exp) -> exactly one
  ~2.7us ACT table load for the whole kernel instead of one per
  sigmoid<->exp alternation.
  xy   = sigmoid(p)*stride + grid*stride   (grid*stride precomputed on host)
  wh   = exp(p) * anchor
  rest = sigmoid(p)

Output layout: writing the natural (25200, 89) tensor costs one 356-byte DMA
packet per row (25200 tiny packets dominated the first profile at ~75% of all
DMA time). Instead each level is stored as (128, NA, R, 89) where partition p
holds rows {t*128+p : t in 0..R} contiguously — per-partition runs of R*356
bytes — and the host reassembles with a cheap numpy transpose.

Inputs x/w are cast to bf16 on host (halves HBM load traffic; the fp32
accumulate keeps the error ~4e-4 relative to output scale).
"""

import numpy as np
import ml_dtypes

import concourse.bacc as bacc
import concourse.mybir as mybir
import concourse.tile as tile
from concourse.bass_utils import run_bass_kernel_spmd

F32 = mybir.dt.float32
F16 = mybir.dt.float16
AF = mybir.ActivationFunctionType
ALU = mybir.AluOpType

NCORES = 8
NA = 3          # anchors per level
NO = 89         # channels per anchor (80 classes + 5 + 4)
NCOL = NA * NO  # 267
GROUP = 4       # full 128-row hw tiles per PSUM group (4 banks)

LEVELS = [
    dict(C=256,  W=80, HW=6400, stride=8.0,
         anchors=((10.0, 13.0), (16.0, 30.0), (33.0, 23.0))),
    dict(C=512,  W=40, HW=1600, stride=16.0,
         anchors=((30.0, 61.0), (62.0, 45.0), (59.0, 119.0))),
    dict(C=1024, W=20, HW=400,  stride=32.0,
         anchors=((116.0, 90.0), (156.0, 198.0), (373.0, 326.0))),
]


def _ntiles(HW):
    return (HW + 127) // 128


def _groups(HW):
    """[(t0, n_full_tiles, rows_per_tile)]; trailing partial tile is its own group."""
    full, rem = divmod(HW, 128)
    out = []
    t0 = 0
    while t0 < full:
        n = min(GROUP, full - t0)
        out.append((t0, n, 128))
        t0 += n
    if rem:
        out.append((full, 1, rem))
    return out


def _store_chunks(nt):
    """Split nt tiles into ~2 store chunks per anchor for DMA pipelining."""
    if nt <= 16:
        return [(0, nt)]
    h = (nt + 1) // 2
    return [(0, h), (h, nt - h)]


def _build_program(use_bias: bool):
    # Bacc (not raw Bass): its compile() runs move_matmul_waits_to_ldweights +
    # generate_event_semaphores, without which walrus rejects instructions
    # that carry more than one semaphore wait.
    nc = bacc.Bacc("TRN2", target_bir_lowering=False, debug=False)

    dram = {}
    for l, L in enumerate(LEVELS):
        nt = _ntiles(L["HW"])
        dram[f"x{l}"] = nc.dram_tensor(f"x{l}", (L["C"], L["HW"]), F16,
                                       kind="ExternalInput").ap()
        dram[f"wt{l}"] = nc.dram_tensor(f"wt{l}", (L["C"], NCOL), F16,
                                        kind="ExternalInput").ap()
        dram[f"gs{l}"] = nc.dram_tensor(f"gs{l}", (128, nt * 6), F32,
                                        kind="ExternalInput").ap()
        dram[f"am{l}"] = nc.dram_tensor(f"am{l}", (128, nt * 6), F32,
                                        kind="ExternalInput").ap()
        dram[f"y{l}"] = nc.dram_tensor(f"y{l}", (128, NA, nt, NO), F32,
                                       kind="ExternalOutput").ap()
        if use_bias:
            dram[f"b{l}"] = nc.dram_tensor(f"b{l}", (1, NCOL), F32,
                                           kind="ExternalInput").ap()

    with tile.TileContext(nc) as tc:
        with tc.tile_pool(name="consts", bufs=1) as cpool, \
             tc.tile_pool(name="xbuf", bufs=1) as xpool, \
             tc.tile_pool(name="obuf", bufs=1) as opool, \
             tc.tile_pool(name="ps", bufs=2, space="PSUM") as pspool:

            ones_t = None
            if use_bias:
                ones_t = cpool.tile([1, 128], F16, tag="ones", name="ones")
                nc.vector.memset(ones_t[:, :], 1.0)

            for l, L in enumerate(LEVELS):
                C, HW, stride = L["C"], L["HW"], L["stride"]
                KC = C // 128
                nt = _ntiles(HW)

                wts = []
                for kc in range(KC):
                    wt_t = cpool.tile([128, NCOL], F16, tag=f"wt{l}k{kc}",
                                      name=f"wt{l}k{kc}")
                    nc.sync.dma_start(out=wt_t[:, :],
                                      in_=dram[f"wt{l}"][kc * 128:(kc + 1) * 128, :])
                    wts.append(wt_t)

                b_t = None
                if use_bias:
                    b_t = cpool.tile([1, NCOL], F16, tag=f"b{l}", name=f"bt{l}")
                    nc.gpsimd.dma_start(out=b_t[:, :], in_=dram[f"b{l}"][:, :])

                gs_t = cpool.tile([128, nt, NA, 2], F32, tag=f"gs{l}", name=f"gst{l}")
                nc.sync.dma_start(
                    out=gs_t[:, :, :, :],
                    in_=dram[f"gs{l}"].rearrange("p (t a c) -> p t a c", a=NA, c=2))
                am_t = cpool.tile([128, nt, NA, 2], F32, tag=f"am{l}", name=f"amt{l}")
                nc.sync.dma_start(
                    out=am_t[:, :, :, :],
                    in_=dram[f"am{l}"].rearrange("p (t a c) -> p t a c", a=NA, c=2))

                xks = []
                for kc in range(KC):
                    xk = xpool.tile([128, HW], F16, tag=f"x{l}k{kc}",
                                    name=f"xk{l}_{kc}")
                    nc.sync.dma_start(out=xk[:, :],
                                      in_=dram[f"x{l}"][kc * 128:(kc + 1) * 128, :])
                    xks.append(xk)

                # whole level's decoded output stays resident, anchor-major so
                # each (partition, anchor) store run is contiguous; partition p
                # element (a, t, :) is output row hw = t*128+p of anchor a
                ot = opool.tile([128, NA, nt, NO], F32, tag=f"ot{l}", name=f"ot{l}")

                for (t0, ntl, m) in _groups(HW):
                    ps = pspool.tile([128, GROUP, 512], F32, tag="ps",
                                     name=f"ps{l}_{t0}")
                    psf = ps.rearrange("p g x -> p (g x)")
                    for i in range(ntl):
                        t = t0 + i
                        for kc in range(KC):
                            nc.tensor.matmul(
                                psf[0:m, i * 512:i * 512 + NCOL],
                                lhsT=xks[kc][:, t * 128:t * 128 + m],
                                rhs=wts[kc][:, :],
                                start=(kc == 0),
                                stop=(kc == KC - 1 and not use_bias),
                            )
                        if use_bias:
                            nc.tensor.matmul(
                                psf[0:m, i * 512:i * 512 + NCOL],
                                lhsT=ones_t[:, 0:m],
                                rhs=b_t[:, :],
                                start=False,
                                stop=True,
                            )

                    og = ot[0:m, :, t0:t0 + ntl, :]  # (m, NA, ntl, 89)
                    # psum viewed anchor-major to match og's enumeration
                    ps_a = ps[0:m, 0:ntl, 0:NCOL].rearrange(
                        "p g (a c) -> p a g c", a=NA)
                    # t = tanh(0.5 * p); sigmoid(p) = 0.5*t + 0.5
                    nc.scalar.activation(og, ps_a, AF.Tanh, scale=0.5)
                    nc.vector.tensor_scalar(og, og, 1.0, 0.5, ALU.add, ALU.mult)
                    # wh: exp(p) (overwrites the sigmoid values on those cols)
                    nc.scalar.activation(og[:, :, :, 2:4], ps_a[:, :, :, 2:4],
                                         AF.Exp)
                    am_a = am_t[0:m, t0:t0 + ntl].transpose([0, 2, 1, 3])
                    nc.vector.tensor_mul(og[:, :, :, 2:4], og[:, :, :, 2:4], am_a)
                    # xy: sigmoid*stride + grid*stride (STT is 3D-max -> per anchor)
                    for a in range(NA):
                        nc.vector.scalar_tensor_tensor(
                            og[:, a, :, 0:2], og[:, a, :, 0:2], float(stride),
                            gs_t[0:m, t0:t0 + ntl, a, 0:2],
                            op0=ALU.mult, op1=ALU.add)

                for a in range(NA):
                    for (s0, snt) in _store_chunks(nt):
                        src = ot[:, a, s0:s0 + snt, :]
                        dst = dram[f"y{l}"][:, a, s0:s0 + snt, :]
                        nc.sync.dma_start(out=dst, in_=src)
    nc.compile()
    return nc


_PROGS = {}


def _get_prog(use_bias: bool):
    if use_bias not in _PROGS:
        _PROGS[use_bias] = _build_program(use_bias)
    return _PROGS[use_bias]


def _host_consts():
    """Per-level (gs, am) host tensors, layout (128, ntiles*6)."""
    consts = []
    for L in LEVELS:
        HW, W, stride = L["HW"], L["W"], L["stride"]
        nt = _ntiles(HW)
        hw = np.arange(nt * 128)
        gx = (hw % W).astype(np.float32) * stride
        gy = (hw // W).astype(np.float32) * stride
        gx[HW:] = 0.0
        gy[HW:] = 0.0
        # gs[p, t, a, c]: c==0 -> gx, c==1 -> gy at position t*128+p
        gs = np.zeros((128, nt, NA, 2), np.float32)
        gs[:, :, :, 0] = gx.reshape(nt, 128).T[:, :, None]
        gs[:, :, :, 1] = gy.reshape(nt, 128).T[:, :, None]
        am = np.zeros((128, nt, NA, 2), np.float32)
        anc = np.asarray(L["anchors"], np.float32)  # (NA, 2)
        am[:, :, :, :] = anc[None, None, :, :]
        consts.append((np.ascontiguousarray(gs.reshape(128, nt * 6)),
                       np.ascontiguousarray(am.reshape(128, nt * 6))))
    return consts


_CONSTS = None


def _make_in_maps(xs, ws, bs, use_bias):
    global _CONSTS
    if _CONSTS is None:
        _CONSTS = _host_consts()
    wts = [np.ascontiguousarray(w.T.astype(ml_dtypes.bfloat16)) for w in ws]
    xbf = [np.ascontiguousarray(
        x.reshape(NCORES, L["C"], L["HW"]).astype(ml_dtypes.bfloat16))
        for x, L in zip(xs, LEVELS)]
    in_maps = []
    for core in range(NCORES):
        im = {}
        for l, L in enumerate(LEVELS):
            im[f"x{l}"] = xbf[l][core]
            im[f"wt{l}"] = wts[l]
            im[f"gs{l}"], im[f"am{l}"] = _CONSTS[l]
            if use_bias:
                im[f"b{l}"] = np.ascontiguousarray(
                    bs[l].reshape(1, NCOL).astype(ml_dtypes.bfloat16))
        in_maps.append(im)
    return in_maps


def _assemble(results):
    """results[core][f"y{l}"] (128, NA, R, 89) -> (NCORES, 25200, 89) fp32."""
    out = np.empty((NCORES, 25200, NO), np.float32)
    for core in range(NCORES):
        parts = []
        for l, L in enumerate(LEVELS):
            HW = L["HW"]
            nt = _ntiles(HW)
            y = results[core][f"y{l}"]  # (128, NA, nt, 89)
            y = y.transpose(1, 2, 0, 3).reshape(NA, nt * 128, NO)[:, :HW, :]
            parts.append(y.reshape(NA * HW, NO))
        out[core] = np.concatenate(parts, axis=0)
    return out


def _run(x0, x1, x2, w0, b0, w1, b1, w2, b2, **spmd_kwargs):
    xs = [np.asarray(x, dtype=np.float32) for x in (x0, x1, x2)]
    ws = [np.asarray(w, dtype=np.float32) for w in (w0, w1, w2)]
    bs = [np.asarray(b, dtype=np.float32) for b in (b0, b1, b2)]
    use_bias = any(np.any(b != 0) for b in bs)
    in_maps = _make_in_maps(xs, ws, bs, use_bias)
    res = run_bass_kernel_spmd(_get_prog(use_bias), in_maps,
                               core_ids=list(range(NCORES)), **spmd_kwargs)
    return _assemble(res.results), res


def kernel(x0, x1, x2, w0, b0, w1, b1, w2, b2):
    out, _ = _run(x0, x1, x2, w0, b0, w1, b1, w2, b2)
    return out


def kernel_traced(x0, x1, x2, w0, b0, w1, b1, w2, b2):
    """Like kernel() but with NTFF tracing; returns (out, BassKernelResults)."""
    return _run(x0, x1, x2, w0, b0, w1, b1, w2, b2, trace=True)
